# revision 7
# baseline (speedup 1.0000x reference)
"""GAT layer kernel for Trainium2 (Bass/Tile), data-parallel over batch on 8 cores.

Reference computation per batch b (N=2048 tokens, F=128 features):
    h   = x @ W                                  [N, F]
    s_n = h[n] . w_mlp + b_mlp                   [N]
    e_ij = leaky_relu(s_i + s_j, 0.2)
    attn = softmax(e, axis=j)
    col_n = sum_i attn[i, n]
    out  = leaky_relu(h * col[:, None], 0.2)

Key algebraic restructure used here (avoids all N^2 transcendentals):
    exp(lrelu(t)) = exp(t) if t > 0 else exp(0.2 t), with t = s_i + s_j, so with
    p = exp(s), q = exp(0.2 s), M_ij = [s_i + s_j > 0] (symmetric 0/1 mask):
        rowsum D_i  = p_i * (M p)_i + q_i * (sum(q) - (M q)_i)
        col_n       = p_n * (M r)_n + q_n * (sum(u) - (M u)_n),  r = p/D, u = q/D
    M is built in bf16 (exact 0/1) with DVE compare ops at 4x rate; the masked
    matvecs run on the PE with [p_hi, p_lo, q_hi, q_lo] bf16 hi/lo split
    stationary vectors for near-fp32 accuracy. col_n > 0 always, so the final
    leaky_relu commutes: out = lrelu(h) * col.
"""

import sys

if "/opt/trn_rl_repo" not in sys.path:
    sys.path.insert(0, "/opt/trn_rl_repo")

from contextlib import ExitStack

import numpy as np

import concourse.bass as bass
import concourse.mybir as mybir
import concourse.tile as tile
from concourse import bacc
from concourse import masks
from concourse.bass_utils import run_bass_kernel_spmd

B, N, F = 8, 2048, 128
NB = N // 128  # 16 token blocks
NEG_SLOPE = 0.2
FP32 = mybir.dt.float32
BF16 = mybir.dt.bfloat16
ALU = mybir.AluOpType
AFT = mybir.ActivationFunctionType


def gat_kernel(ctx: ExitStack, tc: "tile.TileContext", out_d, x_d, W_d, wm_d, bm_d):
    nc = tc.nc

    # ---------------- pools ----------------
    const_p = ctx.enter_context(tc.tile_pool(name="const", bufs=1))
    big_p = ctx.enter_context(tc.tile_pool(name="big", bufs=1))
    mask_p = ctx.enter_context(tc.tile_pool(name="mask", bufs=NB))
    vec_p = ctx.enter_context(tc.tile_pool(name="vec", bufs=1))
    outsb_p = ctx.enter_context(tc.tile_pool(name="outsb", bufs=4))
    # PSUM budget is 8 banks of 2KB/partition. ps_big holds one 4-bank tensor
    # (hT, then d_ps, then g_ps); ps_tr/ps_sm are single-tag pools of 1-bank
    # slots (transposes and small matmul outputs).
    ps_big = ctx.enter_context(tc.tile_pool(name="ps_big", bufs=1, space="PSUM"))
    ps_tr = ctx.enter_context(tc.tile_pool(name="ps_tr", bufs=2, space="PSUM"))
    ps_sm = ctx.enter_context(tc.tile_pool(name="ps_sm", bufs=2, space="PSUM"))

    # ---------------- constants ----------------
    ident_f = const_p.tile([128, 128], FP32, tag="ident_f")
    ident_b = const_p.tile([128, 128], BF16, tag="ident_b")
    masks.make_identity(nc, ident_f[:])
    masks.make_identity(nc, ident_b[:])
    ones_f = const_p.tile([128, 1], FP32, tag="ones_f")
    nc.gpsimd.memset(ones_f[:], 1.0)
    ones_row_f = const_p.tile([1, 128], FP32, tag="ones_row_f")
    nc.gpsimd.memset(ones_row_f[:], 1.0)
    ones_row_b = const_p.tile([1, 128], BF16, tag="ones_row_b")
    nc.gpsimd.memset(ones_row_b[:], 1.0)

    # ---------------- load inputs ----------------
    W_sb = const_p.tile([128, 128], FP32, tag="W_sb")
    nc.sync.dma_start(W_sb[:], W_d[:, :])
    wm_sb = const_p.tile([128, 1], FP32, tag="wm_sb")
    nc.sync.dma_start(wm_sb[:], wm_d.rearrange("(p o) -> p o", o=1))
    b_sb = const_p.tile([1, 1], FP32, tag="b_sb")
    nc.sync.dma_start(b_sb[:], bm_d.rearrange("(p o) -> p o", o=1))
    b_ps = ps_sm.tile([128, 1], FP32, tag="sm")
    nc.tensor.matmul(b_ps[:], lhsT=ones_row_f[:], rhs=b_sb[:], start=True, stop=True)
    b_bc = const_p.tile([128, 1], FP32, tag="b_bc")
    nc.vector.tensor_copy(b_bc[:], b_ps[:])

    # x: [N, F] -> tiles [128 tok, 16, 128 f]
    x_view = x_d.rearrange("(t p) f -> p t f", p=128)
    x_sb = big_p.tile([128, NB, 128], FP32, tag="x_sb")
    for t in range(NB):
        nc.sync.dma_start(x_sb[:, t, :], x_view[:, t, :])

    # ---------------- xT via PE transposes ----------------
    xT = big_p.tile([128, N], FP32, tag="xT")  # [f, tok]
    for t in range(NB):
        xt_ps = ps_tr.tile([128, 128], FP32, tag="tr")
        nc.tensor.transpose(xt_ps[:], x_sb[:, t, :], ident_f[:])
        if t % 2 == 0:
            nc.vector.tensor_copy(xT[:, t * 128 : (t + 1) * 128], xt_ps[:])
        else:
            nc.scalar.copy(xT[:, t * 128 : (t + 1) * 128], xt_ps[:])

    # ---------------- hT = W.T-contracted: [g, tok] ----------------
    hT_ps = ps_big.tile([128, N], FP32, tag="bigps")
    for c in range(4):
        nc.tensor.matmul(
            hT_ps[:, c * 512 : (c + 1) * 512],
            lhsT=W_sb[:],
            rhs=xT[:, c * 512 : (c + 1) * 512],
            start=True,
            stop=True,
        )
    hT_sb = big_p.tile([128, N], FP32, tag="hT_sb")
    for c in range(4):
        sl = slice(c * 512, (c + 1) * 512)
        if c % 2 == 0:
            nc.vector.tensor_copy(hT_sb[:, sl], hT_ps[:, sl])
        else:
            nc.scalar.copy(hT_sb[:, sl], hT_ps[:, sl])

    # ---------------- s in [128, 16] layout ----------------
    # s_mat[p, t] = s[t*128 + p] = sum_g hT[g, t*128+p] * w_mlp[g]
    s_ps = ps_sm.tile([128, NB], FP32, tag="sm")
    for t in range(NB):
        nc.tensor.matmul(
            s_ps[:, t : t + 1],
            lhsT=hT_sb[:, t * 128 : (t + 1) * 128],
            rhs=wm_sb[:],
            start=True,
            stop=True,
        )
    s_mat = vec_p.tile([128, NB], FP32, tag="s_mat")
    nc.vector.tensor_scalar(s_mat[:], s_ps[:], b_bc[:, 0:1], None, ALU.add)
    neg_s = vec_p.tile([128, NB], FP32, tag="neg_s")
    nc.vector.tensor_scalar(neg_s[:], s_mat[:], -1.0, None, ALU.mult)
    s_hi = vec_p.tile([128, NB], BF16, tag="s_hi")
    nc.vector.tensor_copy(s_hi[:], s_mat[:])

    # p = exp(s), q = exp(0.2 s), hi/lo bf16 splits
    p_v = vec_p.tile([128, NB], FP32, tag="p_v")
    nc.scalar.activation(p_v[:], s_mat[:], AFT.Exp)
    q_v = vec_p.tile([128, NB], FP32, tag="q_v")
    nc.scalar.activation(q_v[:], s_mat[:], AFT.Exp, scale=NEG_SLOPE)

    def hi_lo(src, tagbase):
        hi = vec_p.tile([128, NB], BF16, tag=tagbase + "_hi")
        nc.vector.tensor_copy(hi[:], src[:])
        hi32 = vec_p.tile([128, NB], FP32, tag=tagbase + "_hi32")
        nc.vector.tensor_copy(hi32[:], hi[:])
        lo = vec_p.tile([128, NB], BF16, tag=tagbase + "_lo")
        nc.vector.tensor_tensor(lo[:], src[:], hi32[:], ALU.subtract)
        return hi, lo

    p_hi, p_lo = hi_lo(p_v, "p")
    q_hi, q_lo = hi_lo(q_v, "q")

    Pk = vec_p.tile([128, NB, 4], BF16, tag="Pk")
    for c, v in enumerate((p_hi, p_lo, q_hi, q_lo)):
        nc.vector.tensor_copy(Pk[:, :, c], v[:])

    # ---------------- S_row broadcast [128, 2048] bf16 ----------------
    sT_ps = ps_sm.tile([16, 128], BF16, tag="sm")
    nc.tensor.transpose(sT_ps[:], s_hi[:], ident_b[:])
    sT_sb = vec_p.tile([16, 128], BF16, tag="sT_sb")
    nc.vector.tensor_copy(sT_sb[:], sT_ps[:])
    s_flat = vec_p.tile([1, N], BF16, tag="s_flat")
    nc.sync.dma_start(s_flat[0:1, :], sT_sb[:, :])
    S_ps = ps_big.tile([128, N], FP32, tag="bigps")
    for c in range(4):
        sl = slice(c * 512, (c + 1) * 512)
        nc.tensor.matmul(
            S_ps[:, sl], lhsT=ones_row_b[:], rhs=s_flat[0:1, sl], start=True, stop=True
        )
    S_row = big_p.tile([128, N], BF16, tag="S_row")
    for c in range(4):
        sl = slice(c * 512, (c + 1) * 512)
        if c % 2 == 0:
            nc.vector.tensor_copy(S_row[:, sl], S_ps[:, sl])
        else:
            nc.scalar.copy(S_row[:, sl], S_ps[:, sl])

    # ---------------- masks M_a = [s_j > -s_i], bf16 0/1 ----------------
    mask_tiles = []
    for a in range(NB):
        m = mask_p.tile([128, N], BF16, tag="mask")
        nc.vector.tensor_scalar(m[:], S_row[:], neg_s[:, a : a + 1], None, ALU.is_gt)
        mask_tiles.append(m)

    # ---------------- matvec 1: D-parts = sum_j M_ij * [p_hi,p_lo,q_hi,q_lo]_j ----
    d_ps = ps_big.tile([4, N], FP32, tag="bigps")
    for a in range(NB):
        for c in range(4):
            nc.tensor.matmul(
                d_ps[:, c * 512 : (c + 1) * 512],
                lhsT=Pk[:, a, :],
                rhs=mask_tiles[a][:, c * 512 : (c + 1) * 512],
                start=(a == 0),
                stop=(a == NB - 1),
            )
    d_sb = vec_p.tile([4, N], FP32, tag="d_sb")
    for c in range(2):
        sl = slice(c * 1024, (c + 1) * 1024)
        if c == 0:
            nc.vector.tensor_copy(d_sb[:, sl], d_ps[:, sl])
        else:
            nc.scalar.copy(d_sb[:, sl], d_ps[:, sl])

    Dp = vec_p.tile([128, NB, 4], FP32, tag="Dp")
    for t in range(NB):
        dtp = ps_tr.tile([128, 128], FP32, tag="tr")
        nc.tensor.transpose(dtp[:, 0:4], d_sb[:, t * 128 : (t + 1) * 128], ident_f[0:4, 0:4])
        if t % 2 == 0:
            nc.vector.tensor_copy(Dp[:, t, :], dtp[:, 0:4])
        else:
            nc.scalar.copy(Dp[:, t, :], dtp[:, 0:4])

    A_v = vec_p.tile([128, NB], FP32, tag="A_v")
    nc.vector.tensor_tensor(A_v[:], Dp[:, :, 0], Dp[:, :, 1], ALU.add)
    MQ = vec_p.tile([128, NB], FP32, tag="MQ")
    nc.vector.tensor_tensor(MQ[:], Dp[:, :, 2], Dp[:, :, 3], ALU.add)

    # Qtot = sum_j q_j  (reduce free, then partitions via PE, then broadcast)
    def total_of(v, tagbase):
        vs = vec_p.tile([128, 1], FP32, tag=tagbase + "_vs")
        nc.vector.reduce_sum(vs[:], v[:], axis=mybir.AxisListType.X)
        tot_ps = ps_sm.tile([1, 1], FP32, tag="sm")
        nc.tensor.matmul(tot_ps[:], lhsT=vs[:], rhs=ones_f[:], start=True, stop=True)
        tot_sb = vec_p.tile([1, 1], FP32, tag=tagbase + "_totsb")
        nc.vector.tensor_copy(tot_sb[:], tot_ps[:])
        tb_ps = ps_sm.tile([128, 1], FP32, tag="sm")
        nc.tensor.matmul(
            tb_ps[:], lhsT=ones_row_f[:], rhs=tot_sb[:], start=True, stop=True
        )
        tot_bc = vec_p.tile([128, 1], FP32, tag=tagbase + "_totbc")
        nc.vector.tensor_copy(tot_bc[:], tb_ps[:])
        return tot_bc

    q_tot = total_of(q_v, "q")

    # D = p * A + q * (Qtot - MQ)
    dtmp = vec_p.tile([128, NB], FP32, tag="dtmp")
    nc.vector.tensor_scalar(dtmp[:], MQ[:], q_tot[:, 0:1], -1.0, ALU.subtract, ALU.mult)
    t1 = vec_p.tile([128, NB], FP32, tag="t1")
    nc.vector.tensor_tensor(t1[:], p_v[:], A_v[:], ALU.mult)
    t2 = vec_p.tile([128, NB], FP32, tag="t2")
    nc.vector.tensor_tensor(t2[:], q_v[:], dtmp[:], ALU.mult)
    D_v = vec_p.tile([128, NB], FP32, tag="D_v")
    nc.vector.tensor_tensor(D_v[:], t1[:], t2[:], ALU.add)

    invD = vec_p.tile([128, NB], FP32, tag="invD")
    nc.vector.reciprocal(invD[:], D_v[:])
    r_v = vec_p.tile([128, NB], FP32, tag="r_v")
    nc.vector.tensor_tensor(r_v[:], p_v[:], invD[:], ALU.mult)
    u_v = vec_p.tile([128, NB], FP32, tag="u_v")
    nc.vector.tensor_tensor(u_v[:], q_v[:], invD[:], ALU.mult)

    r_hi, r_lo = hi_lo(r_v, "r")
    u_hi, u_lo = hi_lo(u_v, "u")
    Rk = vec_p.tile([128, NB, 4], BF16, tag="Rk")
    for c, v in enumerate((r_hi, r_lo, u_hi, u_lo)):
        nc.vector.tensor_copy(Rk[:, :, c], v[:])
    u_tot = total_of(u_v, "u")

    # ---------------- matvec 2: G-parts = sum_i M_in * [r_hi,r_lo,u_hi,u_lo]_i ----
    g_ps = ps_big.tile([4, N], FP32, tag="bigps")
    for a in range(NB):
        for c in range(4):
            nc.tensor.matmul(
                g_ps[:, c * 512 : (c + 1) * 512],
                lhsT=Rk[:, a, :],
                rhs=mask_tiles[a][:, c * 512 : (c + 1) * 512],
                start=(a == 0),
                stop=(a == NB - 1),
            )
    g_sb = vec_p.tile([4, N], FP32, tag="g_sb")
    for c in range(2):
        sl = slice(c * 1024, (c + 1) * 1024)
        if c == 0:
            nc.vector.tensor_copy(g_sb[:, sl], g_ps[:, sl])
        else:
            nc.scalar.copy(g_sb[:, sl], g_ps[:, sl])

    Gp = vec_p.tile([128, NB, 4], FP32, tag="Gp")
    for t in range(NB):
        gtp = ps_tr.tile([128, 128], FP32, tag="tr")
        nc.tensor.transpose(gtp[:, 0:4], g_sb[:, t * 128 : (t + 1) * 128], ident_f[0:4, 0:4])
        if t % 2 == 0:
            nc.vector.tensor_copy(Gp[:, t, :], gtp[:, 0:4])
        else:
            nc.scalar.copy(Gp[:, t, :], gtp[:, 0:4])

    Gr = vec_p.tile([128, NB], FP32, tag="Gr")
    nc.vector.tensor_tensor(Gr[:], Gp[:, :, 0], Gp[:, :, 1], ALU.add)
    Gu = vec_p.tile([128, NB], FP32, tag="Gu")
    nc.vector.tensor_tensor(Gu[:], Gp[:, :, 2], Gp[:, :, 3], ALU.add)

    gtmp = vec_p.tile([128, NB], FP32, tag="gtmp")
    nc.vector.tensor_scalar(gtmp[:], Gu[:], u_tot[:, 0:1], -1.0, ALU.subtract, ALU.mult)
    c1 = vec_p.tile([128, NB], FP32, tag="c1")
    nc.vector.tensor_tensor(c1[:], p_v[:], Gr[:], ALU.mult)
    c2 = vec_p.tile([128, NB], FP32, tag="c2")
    nc.vector.tensor_tensor(c2[:], q_v[:], gtmp[:], ALU.mult)
    col = vec_p.tile([128, NB], FP32, tag="col")
    nc.vector.tensor_tensor(col[:], c1[:], c2[:], ALU.add)

    # ---------------- out = lrelu(h) * col ----------------
    # lrelu on hT: max(0.2*h, h) in one DVE op
    lrlT = big_p.tile([128, N], FP32, tag="lrlT")
    for c in range(4):
        sl = slice(c * 512, (c + 1) * 512)
        nc.vector.scalar_tensor_tensor(
            lrlT[:, sl], hT_sb[:, sl], NEG_SLOPE, hT_sb[:, sl], ALU.mult, ALU.max
        )

    out_view = out_d.rearrange("(t p) f -> p t f", p=128)
    for t in range(NB):
        ot_ps = ps_tr.tile([128, 128], FP32, tag="tr")
        nc.tensor.transpose(ot_ps[:], lrlT[:, t * 128 : (t + 1) * 128], ident_f[:])
        o_sb = outsb_p.tile([128, 128], FP32, tag="o_sb")
        if t % 2 == 0:
            nc.vector.tensor_scalar(o_sb[:], ot_ps[:], col[:, t : t + 1], None, ALU.mult)
        else:
            nc.scalar.activation(o_sb[:], ot_ps[:], AFT.Copy, scale=col[:, t : t + 1])
        nc.sync.dma_start(out_view[:, t, :], o_sb[:])


def build_nc(num_devices: int = 8) -> "bass.Bass":
    nc = bacc.Bacc(
        "TRN2", target_bir_lowering=False, debug=False, num_devices=num_devices
    )
    x_d = nc.dram_tensor("x", [N, F], FP32, kind="ExternalInput")
    W_d = nc.dram_tensor("W", [F, F], FP32, kind="ExternalInput")
    wm_d = nc.dram_tensor("w_mlp", [F], FP32, kind="ExternalInput")
    bm_d = nc.dram_tensor("b_mlp", [1], FP32, kind="ExternalInput")
    out_d = nc.dram_tensor("out", [N, F], FP32, kind="ExternalOutput")
    with tile.TileContext(nc) as tc:
        with ExitStack() as ctx:
            gat_kernel(ctx, tc, out_d.ap(), x_d.ap(), W_d.ap(), wm_d.ap(), bm_d.ap())
    nc.compile()
    return nc


_NC_CACHE: dict = {}


def run(x, W, w_mlp, b_mlp, trace=False, **spmd_kwargs):
    x = np.asarray(x, dtype=np.float32)
    W = np.asarray(W, dtype=np.float32)
    w_mlp = np.asarray(w_mlp, dtype=np.float32)
    b_mlp = np.asarray(b_mlp, dtype=np.float32)

    if "nc" not in _NC_CACHE:
        _NC_CACHE["nc"] = build_nc(num_devices=B)
    nc = _NC_CACHE["nc"]

    in_maps = [
        {"x": np.ascontiguousarray(x[b, 0]), "W": W, "w_mlp": w_mlp, "b_mlp": b_mlp}
        for b in range(B)
    ]
    res = run_bass_kernel_spmd(
        nc, in_maps, core_ids=list(range(B)), trace=trace, **spmd_kwargs
    )
    out = np.stack([res.results[b]["out"] for b in range(B)])[:, None]
    return out.astype(np.float32), res


def kernel(x, W, w_mlp, b_mlp):
    out, _ = run(x, W, w_mlp, b_mlp)
    return out
